# revision 5
# baseline (speedup 1.0000x reference)
"""Trainium2 Bass kernel: windowed mean-color similarity.

Input  frames [8, 2048, 64, 64, 3] f32  (B, T, H, W, C), lookup_window=101.
Output [8, 2048, 101] f32:
    mc[b,t]    = mean over (H,W) of frames[b,t]            # [B,T,3]
    idx(t,j)   = max(0, t-50) + j                          # window anchor
    sim[b,t,j] = 1/(1 + ||mc[b,t]-mc[b,clip(idx)]||^2)  if idx < min(T, t+51) else 0

Sharding: data-parallel along B, one batch element per NeuronCore (8 cores).
Windows run along T which is fully local per batch element -> no halo.

Per-core kernel (T=2048 rows of HWC=12288 floats, ~100 MB -> memory-bound):
  phase 1: stream frames in 16 tiles [128, 12288] (one 6.3 MB HWDGE DMA each),
           DVE tensor_reduce over the hw axis (stride-3 innermost view
           [128, 3, 4096]) -> per-channel SUMS [128, 3]; DMA into a padded
           DRAM scratch `mc_pad` (sums, not means - the 1/HW^2 scale is folded
           into phase 2).
  phase 2: per tile, a diagonal access pattern DMA (partition p starts at row
           t0+p-50, 303 contiguous floats) gathers each row's neighbor window
           from mc_pad; DVE computes d = sum_c (ctr-nb)^2, then
           sim = 1/(1 + d/HW^2) and multiplies by a host-precomputed validity
           mask. Tile 0 uses a broadcast AP for rows t<50 (window anchored at 0).
"""

import numpy as np

_B, _T, _H, _W, _C = 8, 2048, 64, 64, 3
_HW = _H * _W              # 4096
_HWC = _HW * _C            # 12288
_WL = 101                  # lookup window
_HALF = _WL // 2           # 50
_P = 128                   # SBUF partitions per tile
_NT = _T // _P             # 16 tiles


def _build_nc(T, HW, C, WL, fbufs=3):
    """Build the single-core Bass program (parametrized for small-size sim tests)."""
    import bass_rust
    import concourse.bass as bass
    import concourse.mybir as mybir
    import concourse.tile as tile
    from concourse import bacc

    f32 = mybir.dt.float32
    HWC = HW * C
    HALF = WL // 2
    P = _P
    NT = T // P
    assert T % P == 0 and HALF < P
    PAD_T = T + ((HALF + 63) // 64) * 64   # rows beyond T are zeroed, never valid
    WLC = WL * C

    nc = bacc.Bacc("TRN2")
    frames = nc.dram_tensor("frames", [T, HWC], f32, kind="ExternalInput")
    maskin = nc.dram_tensor("mask", [T, WL], f32, kind="ExternalInput")
    out = nc.dram_tensor("out", [T, WL], f32, kind="ExternalOutput")
    mc_pad = nc.dram_tensor("mc_pad", [PAD_T * C], f32)

    def diag_src(offset_elems, nrows):
        # [nrows, WLC] view of mc_pad: row r starts at offset_elems + r*C
        # (overlapping windows -> custom AP, not expressible via rearrange)
        ap = mc_pad[:].copy()
        ap.ap = bass_rust.VecI64Pair([(C, nrows), (1, WLC)])
        ap.offset = offset_elems
        return ap

    X = mybir.AxisListType.X
    ADD = mybir.AluOpType.add
    SUB = mybir.AluOpType.subtract
    MULT = mybir.AluOpType.mult

    with tile.TileContext(nc) as tc:
        with (
            tc.tile_pool(name="fp", bufs=fbufs) as fp,
            tc.tile_pool(name="mcp", bufs=NT) as mcp,
            tc.tile_pool(name="p2", bufs=3) as p2,
        ):
            # zero the pad tail of mc_pad once (1-partition SBUF->DRAM DMAs
            # fail NEFF load here, so use PAD_T-T partitions x C floats)
            zt = p2.tile([PAD_T - T, C], f32, tag="zt")
            nc.vector.memset(zt[:], 0.0)
            nc.sync.dma_start(
                out=mc_pad[T * C:].rearrange("(p c) -> p c", c=C), in_=zt[:]
            )

            # ---- phase 1: per-tile channel sums -> mc_pad ----
            mcts = []
            for k in range(NT):
                ft = fp.tile([P, HWC], f32, tag="ft")
                nc.sync.dma_start(out=ft[:], in_=frames[k * P:(k + 1) * P, :])
                mct = mcp.tile([P, C], f32, tag="mc")
                v = ft[:].rearrange("p (hw c) -> p c hw", c=C)
                nc.vector.tensor_reduce(out=mct[:], in_=v, axis=X, op=ADD)
                dst = mc_pad[k * P * C:(k + 1) * P * C].rearrange("(p c) -> p c", c=C)
                nc.sync.dma_start(out=dst, in_=mct[:])
                mcts.append(mct)

            # ---- phase 2: windowed similarity ----
            for k in range(NT):
                t0 = k * P
                nb = p2.tile([P, WLC], f32, tag="nb")
                if k == 0:
                    # rows t<HALF: window anchored at row 0 (broadcast)
                    bc = mc_pad[:].copy()
                    bc.ap = bass_rust.VecI64Pair([(0, HALF), (1, WLC)])
                    bc.offset = 0
                    nc.sync.dma_start(out=nb[0:HALF, :], in_=bc)
                    nc.sync.dma_start(out=nb[HALF:P, :], in_=diag_src(0, P - HALF))
                else:
                    nc.sync.dma_start(out=nb[:], in_=diag_src((t0 - HALF) * C, P))

                mct = mcts[k]
                d = p2.tile([P, WLC], f32, tag="d")
                nbv = nb[:].rearrange("p (w c) -> p w c", c=C)
                dv = d[:].rearrange("p (w c) -> p w c", c=C)
                ctr = mct[:].unsqueeze(1).broadcast_to((P, WL, C))
                nc.vector.tensor_tensor(out=dv, in0=ctr, in1=nbv, op=SUB)
                nc.vector.tensor_mul(out=d[:], in0=d[:], in1=d[:])
                dsum = p2.tile([P, WL], f32, tag="dsum")
                nc.vector.tensor_reduce(out=dsum[:], in_=dv, axis=X, op=ADD)
                # sums -> means: diff = dsum/HW^2 ; then +1
                nc.vector.tensor_scalar(
                    out=dsum[:], in0=dsum[:],
                    scalar1=1.0 / (HW * HW), scalar2=1.0, op0=MULT, op1=ADD,
                )
                sim = p2.tile([P, WL], f32, tag="sim")
                nc.vector.reciprocal(out=sim[:], in_=dsum[:])
                mt = p2.tile([P, WL], f32, tag="mt")
                nc.sync.dma_start(out=mt[:], in_=maskin[t0:t0 + P, :])
                nc.vector.tensor_mul(out=sim[:], in0=sim[:], in1=mt[:])
                nc.sync.dma_start(out=out[t0:t0 + P, :], in_=sim[:])

    nc.compile()
    return nc


def _valid_mask(T, WL):
    t = np.arange(T)[:, None]
    j = np.arange(WL)[None, :]
    half = WL // 2
    start = np.maximum(0, t - half)
    end = np.minimum(T, t + half + 1)
    return ((start + j) < end).astype(np.float32)


_NC_CACHE = {}


def kernel(frames, lookup_window):
    frames = np.asarray(frames, dtype=np.float32)
    lookup_window = int(lookup_window)
    assert frames.shape == (_B, _T, _H, _W, _C), frames.shape
    assert lookup_window == _WL, lookup_window

    from concourse.bass_utils import run_bass_kernel_spmd

    if "nc" not in _NC_CACHE:
        _NC_CACHE["nc"] = _build_nc(_T, _HW, _C, _WL)
    nc = _NC_CACHE["nc"]

    mask = _valid_mask(_T, _WL)
    flat = np.ascontiguousarray(frames.reshape(_B, _T, _HWC))
    in_maps = [{"frames": flat[b], "mask": mask} for b in range(_B)]
    res = run_bass_kernel_spmd(nc, in_maps, list(range(_B)))
    return np.stack([res.results[b]["out"] for b in range(_B)], axis=0)


# revision 8
# speedup vs baseline: 12.9634x; 12.9634x over previous
"""Trainium2 Bass kernel: windowed mean-color similarity.

Input  frames [8, 2048, 64, 64, 3] f32  (B, T, H, W, C), lookup_window=101.
Output [8, 2048, 101] f32:
    mc[b,t]    = mean over (H,W) of frames[b,t]            # [B,T,3]
    idx(t,j)   = max(0, t-50) + j                          # window anchor
    sim[b,t,j] = 1/(1 + ||mc[b,t]-mc[b,clip(idx)]||^2)  if idx < min(T, t+51) else 0

Sharding: data-parallel along B, one batch element per NeuronCore (8 cores).
Windows run along T which is fully local per batch element -> no halo.

Per-core kernel (T=2048 rows of HWC=12288 floats, ~100 MB -> memory-bound):
  phase 1: stream frames in 16 tiles [128, 12288] (one 6.3 MB HWDGE DMA each),
           DVE tensor_reduce over the hw axis (stride-3 innermost view
           [128, 3, 4096]) -> per-channel SUMS [128, 3]; DMA into a padded
           DRAM scratch `mc_pad` (sums, not means - the 1/HW^2 scale is folded
           into phase 2).
  phase 2: per tile, a diagonal access pattern DMA (partition p starts at row
           t0+p-50, 303 contiguous floats) gathers each row's neighbor window
           from mc_pad; DVE computes d = sum_c (ctr-nb)^2, then
           sim = 1/(1 + d/HW^2) and multiplies by a host-precomputed validity
           mask. Tile 0 uses a broadcast AP for rows t<50 (window anchored at 0).
"""

import numpy as np

_B, _T, _H, _W, _C = 8, 2048, 64, 64, 3
_HW = _H * _W              # 4096
_HWC = _HW * _C            # 12288
_WL = 101                  # lookup window
_HALF = _WL // 2           # 50
_P = 128                   # SBUF partitions per tile
_NT = _T // _P             # 16 tiles


def _one_pass(nc, fp, mcp, p2, frames, maskin, out, mc_pad, T, HW, C, WL):
    """Emit one full pass (phase 1 + phase 2) into the open TileContext."""
    import bass_rust
    import concourse.mybir as mybir

    f32 = mybir.dt.float32
    HWC = HW * C
    HALF = WL // 2
    P = _P
    NT = T // P
    WLC = WL * C
    X = mybir.AxisListType.X
    ADD = mybir.AluOpType.add
    SUB = mybir.AluOpType.subtract
    MULT = mybir.AluOpType.mult

    def diag_src(offset_elems, nrows):
        # [nrows, WLC] view of mc_pad: row r starts at offset_elems + r*C
        # (overlapping windows -> custom AP, not expressible via rearrange)
        ap = mc_pad[:].copy()
        ap.ap = bass_rust.VecI64Pair([(C, nrows), (1, WLC)])
        ap.offset = offset_elems
        return ap

    # ---- phase 1: per-tile channel sums -> mc_pad ----
    mcts = []
    for k in range(NT):
        ft = fp.tile([P, HWC], f32, tag="ft")
        nc.sync.dma_start(out=ft[:], in_=frames[k * P:(k + 1) * P, :])
        mct = mcp.tile([P, C], f32, tag="mc")
        v = ft[:].rearrange("p (hw c) -> p c hw", c=C)
        nc.vector.tensor_reduce(out=mct[:], in_=v, axis=X, op=ADD)
        dst = mc_pad[k * P * C:(k + 1) * P * C].rearrange("(p c) -> p c", c=C)
        nc.sync.dma_start(out=dst, in_=mct[:])
        mcts.append(mct)

    # ---- phase 2: windowed similarity ----
    for k in range(NT):
        t0 = k * P
        nb = p2.tile([P, WLC], f32, tag="nb")
        if k == 0:
            # rows t<HALF: window anchored at row 0 (broadcast)
            bc = mc_pad[:].copy()
            bc.ap = bass_rust.VecI64Pair([(0, HALF), (1, WLC)])
            bc.offset = 0
            nc.sync.dma_start(out=nb[0:HALF, :], in_=bc)
            nc.sync.dma_start(out=nb[HALF:P, :], in_=diag_src(0, P - HALF))
        else:
            nc.sync.dma_start(out=nb[:], in_=diag_src((t0 - HALF) * C, P))

        mct = mcts[k]
        d = p2.tile([P, WLC], f32, tag="d")
        nbv = nb[:].rearrange("p (w c) -> p w c", c=C)
        dv = d[:].rearrange("p (w c) -> p w c", c=C)
        ctr = mct[:].unsqueeze(1).broadcast_to((P, WL, C))
        nc.vector.tensor_tensor(out=dv, in0=ctr, in1=nbv, op=SUB)
        nc.vector.tensor_mul(out=d[:], in0=d[:], in1=d[:])
        dsum = p2.tile([P, WL], f32, tag="dsum")
        nc.vector.tensor_reduce(out=dsum[:], in_=dv, axis=X, op=ADD)
        # sums -> means: diff = dsum/HW^2 ; then +1
        nc.vector.tensor_scalar(
            out=dsum[:], in0=dsum[:],
            scalar1=1.0 / (HW * HW), scalar2=1.0, op0=MULT, op1=ADD,
        )
        sim = p2.tile([P, WL], f32, tag="sim")
        nc.vector.reciprocal(out=sim[:], in_=dsum[:])
        mt = p2.tile([P, WL], f32, tag="mt")
        nc.sync.dma_start(out=mt[:], in_=maskin[t0:t0 + P, :])
        nc.vector.tensor_mul(out=sim[:], in0=sim[:], in1=mt[:])
        nc.sync.dma_start(out=out[t0:t0 + P, :], in_=sim[:])


def _build_nc(T, HW, C, WL, fbufs=3, reps=1):
    """Build the single-core Bass program (parametrized for small-size sim
    tests). reps>1 repeats the computation back-to-back inside one NEFF —
    benchmarking only (amortizes the ~3 ms axon dispatch RTT)."""
    import concourse.mybir as mybir
    import concourse.tile as tile
    from concourse import bacc

    f32 = mybir.dt.float32
    HWC = HW * C
    HALF = WL // 2
    P = _P
    NT = T // P
    assert T % P == 0 and HALF < P
    PAD_T = T + ((HALF + 63) // 64) * 64   # rows beyond T are zeroed, never valid

    nc = bacc.Bacc("TRN2")
    frames = nc.dram_tensor("frames", [T, HWC], f32, kind="ExternalInput")
    maskin = nc.dram_tensor("mask", [T, WL], f32, kind="ExternalInput")
    out = nc.dram_tensor("out", [T, WL], f32, kind="ExternalOutput")
    mc_pad = nc.dram_tensor("mc_pad", [PAD_T * C], f32)

    with tile.TileContext(nc) as tc:
        with (
            tc.tile_pool(name="fp", bufs=fbufs) as fp,
            tc.tile_pool(name="mcp", bufs=NT) as mcp,
            tc.tile_pool(name="p2", bufs=3) as p2,
        ):
            # zero the pad tail of mc_pad once (1-partition SBUF->DRAM DMAs
            # fail NEFF load here, so use PAD_T-T partitions x C floats)
            zt = p2.tile([PAD_T - T, C], f32, tag="zt")
            nc.vector.memset(zt[:], 0.0)
            nc.sync.dma_start(
                out=mc_pad[T * C:].rearrange("(p c) -> p c", c=C), in_=zt[:]
            )
            for _rep in range(reps):
                _one_pass(nc, fp, mcp, p2, frames, maskin, out, mc_pad,
                          T, HW, C, WL)

    nc.compile()
    return nc


def _valid_mask(T, WL):
    t = np.arange(T)[:, None]
    j = np.arange(WL)[None, :]
    half = WL // 2
    start = np.maximum(0, t - half)
    end = np.minimum(T, t + half + 1)
    return ((start + j) < end).astype(np.float32)


_NC_CACHE = {}


def kernel(frames, lookup_window):
    frames = np.asarray(frames, dtype=np.float32)
    lookup_window = int(lookup_window)
    assert frames.shape == (_B, _T, _H, _W, _C), frames.shape
    assert lookup_window == _WL, lookup_window

    from concourse.bass_utils import run_bass_kernel_spmd

    if "nc" not in _NC_CACHE:
        _NC_CACHE["nc"] = _build_nc(_T, _HW, _C, _WL)
    nc = _NC_CACHE["nc"]

    mask = _valid_mask(_T, _WL)
    flat = np.ascontiguousarray(frames.reshape(_B, _T, _HWC))
    in_maps = [{"frames": flat[b], "mask": mask} for b in range(_B)]
    res = run_bass_kernel_spmd(nc, in_maps, list(range(_B)))
    return np.stack([res.results[b]["out"] for b in range(_B)], axis=0)
